# revision 22
# baseline (speedup 1.0000x reference)
"""Trainium2 Bass kernel for nn_Attention_56530359550323.

Full-input contract: kernel(**inputs) takes the unsharded inputs and returns
the full [4, 2048, 4096] float32 output.

Sharding: 8 cores = 4 batches (data-parallel) x 2 head-groups
(tensor-parallel over the 4 query heads; the single kv head is replicated).
Each core computes a partial output-projection [4096, 2048] (transposed);
the host sums the two partials per batch ("all-reduce after wo") and
transposes back.

Device algorithm (everything feature-major / transposed so every matmul has
a 512-wide moving operand, full rate):
  phase 1: qT/kT/vT = W^T @ xT accumulated over 32 d-chunks, with the
           per-d-group weight DMAs interleaved with the x stream so the
           first matmuls start ~5us in.  The rope epilogue for s-block sb
           is emitted after the projection matmuls of sb+1 so its DVE/ACT
           work overlaps PE time: rope is 2 ACT half-swaps + 3
           full-128-partition DVE ops against host-prepared duplicated-cos
           / signed-sin tables.  The 12 per-position sum-of-squares rows
           (4 k + 8 q) are collected into one [12,512] SBUF tile and
           normalized with a single Sqrt -> reciprocal_approx_fast ->
           qscale-mul chain (one pass instead of 12 serial
           single-partition reciprocals).  Q factors are broadcast over
           partitions with K=1 ones matmuls; K factors are NOT applied to
           kT at all -- they are transposed into a [128 kv, 1] per-chunk
           layout and folded into phase 2's Exp as its per-partition
           scale operand (exp(s*f_k) == exp-of-normalized-score).
  phase 2: per (q-block 512, head): scoresT = kT_chunk^T @ qT (chunk=128 kv
           positions), exp on ACT with the per-kv k-norm scale (scores are
           bounded ~6 after qk-norm so no max subtraction is needed),
           causal masking only on the 4 diagonal chunks via constant uint8
           masks, PV accumulation outT += v_chunk^T @ expT, row sums via a
           ones-column matmul, normalization by the broadcast reciprocal
           (approx-fast on SBUF).
  phase 3: partial out-projection outT[cc] = sum_h wo[h,cc]^T @ attnT_h,
           written back as float16 partials (host sums in fp32).
"""

import os
import sys
from contextlib import ExitStack

import numpy as np

if "/opt/trn_rl_repo" not in sys.path:
    sys.path.insert(0, "/opt/trn_rl_repo")

import concourse.bass as bass
import concourse.mybir as mybir
import concourse.tile as tile
from concourse import bacc, bass_utils

# ---- problem constants (hardcoded per contract) ----
B, S, D = 4, 2048, 4096
HEAD_DIM = 128
N_HEADS = 4            # local q heads in the reference module
N_KV = 1
ROPE_THETA = 500000.0
EPS = 1e-6
FLOOR_SCALE = 8192.0
ATTN_SCALE = 0.1

P = 128                # partitions
SB = 512               # s-block (q-block) size
NSB = S // SB          # 4
ND = D // P            # 32 contraction chunks for projections
NKCH = S // P          # 16 kv chunks
NCC = D // P           # 32 output column chunks
HG = 2                 # heads per group (tensor-parallel degree 2)
NRN = 3 * NSB          # rope_norm rows: k rows 0..3, q rows 4..11
NQR = HG * NSB         # 8 q rows

f32 = mybir.dt.float32
f32r = mybir.dt.float32r
f16 = mybir.dt.float16
bf16 = mybir.dt.bfloat16
u8 = mybir.dt.uint8

MM_MODE = os.environ.get("KERNEL_MM_MODE", "f16")

_BUILD_CACHE = {}


def _mm_ap(ap):
    if MM_MODE == "f32r" and ap.dtype == f32:
        return ap.bitcast(f32r)
    return ap


def _dram_mm_dt():
    return {"f32r": f32r, "f16": f16, "bf16": bf16}[MM_MODE]


def _tile_mm_dt():
    return {"f32r": f32r, "f16": f16, "bf16": bf16}[MM_MODE]


def _np_mm_dt():
    if MM_MODE == "f32r":
        return np.float32
    if MM_MODE == "f16":
        return np.float16
    import ml_dtypes

    return ml_dtypes.bfloat16


def build_bass():
    key = MM_MODE
    if key in _BUILD_CACHE:
        return _BUILD_CACHE[key]

    wdt = _dram_mm_dt()      # dram/sbuf dtype for x/w (matmul-only tensors)
    tdt = _tile_mm_dt()      # sbuf dtype for DVE/ACT-produced matmul inputs

    nc = bacc.Bacc("TRN2", target_bir_lowering=False, debug=False)

    # all big tensors arrive pre-tiled host-side so every DMA is a
    # contiguous per-partition read (avoids the 256B-1KB descriptor storm)
    xT_d = nc.dram_tensor("xT", (NSB, 8, P, 4, SB), wdt, kind="ExternalInput").ap()
    wq_d = nc.dram_tensor("wq_g", (8, P, 4, HG * HEAD_DIM), wdt, kind="ExternalInput").ap()
    wk_d = nc.dram_tensor("wk", (8, P, 4, HEAD_DIM), wdt, kind="ExternalInput").ap()
    wv_d = nc.dram_tensor("wv", (8, P, 4, HEAD_DIM), wdt, kind="ExternalInput").ap()
    wo_d = nc.dram_tensor("wo_g", (P, HG, NCC, P), wdt, kind="ExternalInput").ap()
    # duplicated cos [cos; cos] and signed sin [-sin; +sin], feature-major
    csD_d = nc.dram_tensor("csD", (P, S), f32, kind="ExternalInput").ap()
    snS_d = nc.dram_tensor("snS", (P, S), f32, kind="ExternalInput").ap()
    # per-norm-row scale grid: k rows (0..3) = 1, q rows (4..11) = qscale
    qsg_d = nc.dram_tensor("qsg", (NRN, SB), f32, kind="ExternalInput").ap()
    out_d = nc.dram_tensor("outT", (NCC, NSB, P, SB), f16, kind="ExternalOutput").ap()

    # masks for the 4 diagonal chunks of a 512-q block: 1 => future (kill)
    masks_np = np.zeros((P, 4, SB), np.uint8)
    for c in range(4):
        kp = c * P + np.arange(P)[:, None]
        qf = np.arange(SB)[None, :]
        masks_np[:, c, :] = (kp > qf).astype(np.uint8)
    masks_d = nc.inline_tensor(masks_np, name="cmasks")
    ident_d = nc.inline_tensor(np.eye(P, dtype=_np_mm_dt()), name="ident")
    ident4_d = nc.inline_tensor(np.eye(4, dtype=_np_mm_dt()), name="ident4")
    # esel[:, r, j] = (j == r): ones-column matmul that drops the sum-of-
    # squares of row r into partition r of a shared [12,512] PSUM tile
    esel_np = np.broadcast_to(
        np.eye(NRN, dtype=_np_mm_dt()), (P, NRN, NRN)
    ).copy()
    esel_d = nc.inline_tensor(esel_np, name="esel")
    # qsel[p, r, :] = (p == 4 + r): broadcasts row 4+r of the norm-factor
    # tile to all 128 partitions in a single K=12 matmul
    qsel_np = np.zeros((NRN, NQR, P), _np_mm_dt())
    for r in range(NQR):
        qsel_np[4 + r, r, :] = 1
    qsel_d = nc.inline_tensor(qsel_np, name="qsel")

    Exp = mybir.ActivationFunctionType.Exp
    Sqrt = mybir.ActivationFunctionType.Sqrt
    Square = mybir.ActivationFunctionType.Square

    with tile.TileContext(nc) as tc, ExitStack() as top:
        cpool = top.enter_context(tc.tile_pool(name="consts", bufs=1))
        qkpool = top.enter_context(tc.tile_pool(name="qkv", bufs=1))
        atpool = top.enter_context(tc.tile_pool(name="attn", bufs=1))

        masks_t = cpool.tile([P, 4, SB], u8)
        nc.gpsimd.dma_start(masks_t, masks_d.ap())
        ident_t = cpool.tile([P, P], tdt)
        nc.gpsimd.dma_start(ident_t, ident_d.ap())
        ident4_t = cpool.tile([4, 4], tdt)
        nc.gpsimd.dma_start(ident4_t, ident4_d.ap())
        esel_t = cpool.tile([P, NRN, NRN], tdt)
        nc.gpsimd.dma_start(esel_t, esel_d.ap())
        qsel_t = cpool.tile([NRN, NQR, P], tdt)
        nc.gpsimd.dma_start(qsel_t, qsel_d.ap())
        onesrow_t = cpool.tile([1, P], tdt)
        nc.vector.memset(onesrow_t, 1.0)
        onescol_t = cpool.tile([P, 1], tdt)
        nc.vector.memset(onescol_t, 1.0)
        zero_t = cpool.tile([P, SB], tdt)
        nc.vector.memset(zero_t, 0.0)
        eps12_t = cpool.tile([NRN, 1], f32)
        nc.vector.memset(eps12_t, float(EPS))

        # cross-phase SBUF handoff tiles
        qT_t = qkpool.tile([P, HG, S], tdt)       # normed+roped+scaled qT
        kT_t = qkpool.tile([P, S], tdt)           # roped (UN-normed) kT
        fkT_t = qkpool.tile([P, NSB, 4], f32)     # k norm factor, kv-partition-major
        vnat_t = qkpool.tile([P, NKCH, P], tdt)   # v in natural [s, hd] tiles
        attnT_t = atpool.tile([P, HG, S], tdt)

        # ---------------- phase 1: projections ----------------
        with ExitStack() as ph1:
            rcpool = ph1.enter_context(tc.tile_pool(name="ropec", bufs=1))
            wpool = ph1.enter_context(tc.tile_pool(name="projw", bufs=1))
            xpool = ph1.enter_context(tc.tile_pool(name="xstream", bufs=4))
            epool = ph1.enter_context(tc.tile_pool(name="ep1", bufs=2))
            tpool = ph1.enter_context(tc.tile_pool(name="ropetmp", bufs=2))
            p1ps = ph1.enter_context(tc.tile_pool(name="p1ps", bufs=1, space="PSUM"))
            p1ps2 = ph1.enter_context(tc.tile_pool(name="p1ps2", bufs=1, space="PSUM"))

            csD_t = rcpool.tile([P, S], f32)
            snS_t = rcpool.tile([P, S], f32)
            qsg_t = rcpool.tile([NRN, SB], f32)
            ropes = rcpool.tile([P, NQR, SB], tdt)  # roped (un-normed) q rows
            # all 12 sum-of-squares rows accumulate into one PSUM tile
            ss_all_ps = p1ps2.tile([NRN, SB], f32, tag="ss")
            ss_count = [0]

            wq_t = wpool.tile([P, ND, HG * HEAD_DIM], wdt)
            wk_t = wpool.tile([P, ND, HEAD_DIM], wdt)
            wv_t = wpool.tile([P, ND, HEAD_DIM], wdt)

            def epilogue_a(sb, copies, kc, v_sb):
                """rope + square + sum-of-squares for s-block sb; v transpose."""
                ss = slice(sb * SB, (sb + 1) * SB)
                for j, srcc in enumerate((copies[0], copies[1], kc)):
                    is_k = j == 2
                    r = sb if is_k else 4 + sb * HG + j
                    rope_dst = kT_t[:, ss] if is_k else ropes[:, r - 4, :]
                    swp = tpool.tile([P, SB], f32, tag="swp")
                    nc.scalar.copy(swp[0:64, :], srcc[64:128, :])
                    nc.scalar.copy(swp[64:128, :], srcc[0:64, :])
                    t1 = tpool.tile([P, SB], f32, tag="t1")
                    nc.vector.tensor_mul(t1, srcc, csD_t[:, ss])
                    t2 = tpool.tile([P, SB], f32, tag="t2")
                    nc.vector.tensor_mul(t2, swp, snS_t[:, ss])
                    nc.vector.tensor_add(rope_dst, t1, t2)
                    sq = tpool.tile([P, SB], tdt, tag="sq")
                    nc.scalar.activation(sq, rope_dst, Square)
                    nc.tensor.matmul(
                        ss_all_ps, _mm_ap(esel_t[:, r, :]), _mm_ap(sq[:]),
                        start=(ss_count[0] == 0),
                        stop=(ss_count[0] == NRN - 1),
                    )
                    ss_count[0] += 1
                for t in range(4):
                    tp_ps = p1ps2.tile([P, P], tdt, tag="tp")
                    nc.tensor.transpose(tp_ps, v_sb[:, t * P:(t + 1) * P], _mm_ap(ident_t[:]))
                    nc.vector.tensor_copy(vnat_t[:, sb * 4 + t, :], tp_ps)

            deferred = []
            for sb in range(NSB):
                q_ps = [
                    p1ps.tile([P, SB], f32, tag=f"q{h}", name=f"qps{h}")
                    for h in range(HG)
                ]
                k_ps = p1ps.tile([P, SB], f32, tag="k")
                v_ps = p1ps.tile([P, SB], f32, tag="v")
                for dg in range(8):
                    dsl = slice(dg * 4, (dg + 1) * 4)
                    if sb == 0:
                        # interleave weight loads with the x stream so the
                        # first d-group's matmuls start ~5us in
                        nc.sync.dma_start(wq_t[:, dsl, :], wq_d[dg])
                        nc.scalar.dma_start(wk_t[:, dsl, :], wk_d[dg])
                        nc.gpsimd.dma_start(wv_t[:, dsl, :], wv_d[dg])
                        if dg == 0:
                            nc.gpsimd.dma_start(csD_t, csD_d)
                            nc.gpsimd.dma_start(snS_t, snS_d)
                            nc.gpsimd.dma_start(qsg_t, qsg_d)
                    xt = xpool.tile([P, 4, SB], wdt)
                    if sb == 0 and dg == 0:
                        # split the first x tile so the first matmul's
                        # operand lands as early as possible
                        for c4 in range(4):
                            nc.sync.dma_start(
                                xt[:, c4, :], xT_d[sb, dg, :, c4, :]
                            )
                    else:
                        nc.sync.dma_start(xt, xT_d[sb, dg])
                    for c in range(4):
                        d = dg * 4 + c
                        st, sp = (d == 0), (d == ND - 1)
                        rhs = _mm_ap(xt[:, c, :])
                        for h in range(HG):
                            nc.tensor.matmul(
                                q_ps[h],
                                _mm_ap(wq_t[:, d, h * P:(h + 1) * P]),
                                rhs,
                                start=st,
                                stop=sp,
                            )
                        nc.tensor.matmul(
                            k_ps, _mm_ap(wk_t[:, d, :]), rhs, start=st, stop=sp
                        )
                        nc.tensor.matmul(
                            v_ps, _mm_ap(wv_t[:, d, :]), rhs, start=st, stop=sp
                        )

                # free the accumulator banks fast (ACT reads PSUM at line rate)
                copies = []
                for h in range(HG):
                    qc = epool.tile([P, SB], f32, tag=f"qc{h}", bufs=2,
                                    name=f"qcopy{h}")
                    nc.scalar.copy(qc, q_ps[h])
                    copies.append(qc)
                kc = epool.tile([P, SB], f32, tag="kc", bufs=2)
                nc.scalar.copy(kc, k_ps)
                v_sb = epool.tile([P, SB], tdt, tag="vc", bufs=2)
                nc.vector.tensor_copy(v_sb, v_ps)
                deferred.append((sb, copies, kc, v_sb))
                # emit the previous block's epilogue after this block's
                # matmuls: its DVE/ACT work overlapped this block's PE time
                if len(deferred) > 1:
                    epilogue_a(*deferred.pop(0))
            epilogue_a(*deferred.pop(0))

            # batched normalization chain: one pass over all 12 rows
            sqr_all = epool.tile([NRN, SB], f32, tag="sqr", bufs=1)
            nc.scalar.activation(
                sqr_all, ss_all_ps, Sqrt, bias=eps12_t[:], scale=1.0 / HEAD_DIM
            )
            rec_all = epool.tile([NRN, SB], f32, tag="reca", bufs=1)
            nc.vector.reciprocal_approx_fast(rec_all, sqr_all)
            recq_all = epool.tile([NRN, SB], f32, tag="recq", bufs=1)
            nc.vector.tensor_mul(recq_all, rec_all, qsg_t)
            recr_all = epool.tile([NRN, SB], tdt, tag="recr", bufs=1)
            nc.vector.tensor_copy(recr_all, recq_all)

            # k factors -> [128 kv, 1] per-chunk layout via 4 PE transposes
            for m in range(4):
                tp_ps = p1ps2.tile([P, P], tdt, tag="tp")
                nc.tensor.transpose(
                    tp_ps[:, 0:4], recr_all[0:4, m * P:(m + 1) * P], ident4_t[:]
                )
                nc.vector.tensor_copy(fkT_t[:, :, m], tp_ps[:, 0:4])

            # q factors: selector matmul broadcasts row 4+r to all
            # partitions in one shot (reads the full base-0 tile)
            for r in range(NQR):
                sb, j = divmod(r, HG)
                ss = slice(sb * SB, (sb + 1) * SB)
                bc_ps = p1ps2.tile([P, SB], f32, tag="bc", bufs=2)
                nc.tensor.matmul(
                    bc_ps, _mm_ap(qsel_t[:, r, :]), _mm_ap(recr_all[:]),
                    start=True, stop=True,
                )
                bcs = epool.tile([P, SB], f32, tag="bcs", bufs=2)
                nc.scalar.copy(bcs, bc_ps)
                nc.vector.tensor_mul(qT_t[:, j, ss], ropes[:, r, :], bcs)

        # ---------------- phase 2: attention ----------------
        with ExitStack() as ph2:
            e2pool = ph2.enter_context(tc.tile_pool(name="ep2", bufs=2))
            wopool = ph2.enter_context(tc.tile_pool(name="wo", bufs=1))
            o3pool = ph2.enter_context(tc.tile_pool(name="oc", bufs=8))
            expool = ph2.enter_context(tc.tile_pool(name="exps", bufs=12))
            p2ps = ph2.enter_context(tc.tile_pool(name="p2ps", bufs=2, space="PSUM"))
            p2sc = ph2.enter_context(tc.tile_pool(name="p2sc", bufs=1, space="PSUM"))

            def finalize(h, qs_sl, pv_ps, rs_ps):
                # normalization chain, emitted one group late so its PE
                # broadcast matmul never stalls the in-order PE stream.
                pvs = e2pool.tile([P, SB], f32, tag="pvs")
                nc.scalar.copy(pvs, pv_ps)
                rss = e2pool.tile([1, SB], f32, tag="rss")
                nc.scalar.copy(rss, rs_ps)
                rec = e2pool.tile([1, SB], f32, tag="rec")
                nc.vector.reciprocal_approx_fast(rec, rss)
                recr = e2pool.tile([1, SB], tdt, tag="recr")
                nc.vector.tensor_copy(recr, rec)
                bc_ps = p2sc.tile([P, SB], f32, tag="obc", bufs=2)
                nc.tensor.matmul(
                    bc_ps, _mm_ap(onesrow_t[:]), _mm_ap(recr[:]),
                    start=True, stop=True,
                )
                bc_sb = e2pool.tile([P, SB], f32, tag="bcc")
                nc.vector.tensor_copy(bc_sb, bc_ps)
                nc.vector.tensor_mul(attnT_t[:, h, qs_sl], pvs, bc_sb)

            wo_t = wopool.tile([P, HG, NCC, P], wdt)
            nc.sync.dma_start(wo_t, wo_d)

            def outproj(qb):
                qsl = slice(qb * SB, (qb + 1) * SB)
                for cc in range(NCC):
                    o_ps = p2sc.tile([P, SB], f32, tag="obc", bufs=2, name="ops")
                    for h in range(HG):
                        nc.tensor.matmul(
                            o_ps,
                            _mm_ap(wo_t[:, h, cc, :]),
                            _mm_ap(attnT_t[:, h, qsl]),
                            start=(h == 0),
                            stop=(h == HG - 1),
                        )
                    o_sb = o3pool.tile([P, SB], f16, tag="oc")
                    if cc % 2 == 0:
                        nc.vector.tensor_copy(o_sb, o_ps)
                        nc.gpsimd.dma_start(out_d[cc, qb], o_sb)
                    else:
                        nc.scalar.copy(o_sb, o_ps)
                        nc.sync.dma_start(out_d[cc, qb], o_sb)

            def emit_rs(rs_ps, quads):
                # row-sum matmuls for a finished group: its quad sums are
                # long done, so these never stall the PE stream
                for i, eq in enumerate(quads):
                    nc.tensor.matmul(
                        rs_ps, _mm_ap(onescol_t[:]), _mm_ap(eq[:]),
                        start=(i == 0), stop=(i == len(quads) - 1),
                    )

            pending = []
            rs_defer = []
            for qb in range(NSB):
                qs_sl = slice(qb * SB, (qb + 1) * SB)
                nch = 4 * qb + 4
                for h in range(HG):
                    pv_ps = p2ps.tile([P, SB], f32, tag="pv")
                    rs_ps = p2ps.tile([1, SB], f32, tag="rs", bufs=1)
                    qt = qT_t[:, h, qs_sl]
                    if rs_defer:
                        emit_rs(*rs_defer.pop(0))
                    equad = None
                    quads = []
                    exps = []

                    def emit_sc(c):
                        # scores + exp for chunk c, staggered one chunk ahead
                        # of its PV matmul so the PE never waits on the ACT
                        sc_ps = p2sc.tile([P, SB], f32, tag="sc", bufs=3)
                        nc.tensor.matmul(
                            sc_ps,
                            _mm_ap(kT_t[:, c * P:(c + 1) * P]),
                            _mm_ap(qt),
                            start=True,
                            stop=True,
                        )
                        e_sb = expool.tile([P, SB], tdt, tag="exp")
                        nc.scalar.activation(
                            e_sb, sc_ps, Exp, scale=fkT_t[:, c // 4, c % 4:c % 4 + 1]
                        )
                        if c >= 4 * qb:
                            nc.vector.copy_predicated(
                                e_sb, masks_t[:, c - 4 * qb, :], zero_t
                            )
                        exps.append(e_sb)

                    emit_sc(0)
                    for c in range(nch):
                        if c + 1 < nch:
                            emit_sc(c + 1)
                        e_sb = exps[c]
                        st, sp = (c == 0), (c == nch - 1)
                        nc.tensor.matmul(
                            pv_ps, _mm_ap(vnat_t[:, c, :]), _mm_ap(e_sb[:]),
                            start=st, stop=sp,
                        )
                        # accumulate quads of exp tiles on DVE so the row-sum
                        # matmul runs once per 4 chunks instead of per chunk
                        if c % 4 == 0:
                            equad = e_sb
                        else:
                            nb = 8 if c % 4 == 3 else 2
                            eacc = expool.tile([P, SB], tdt, tag=f"ea{c % 4}",
                                               bufs=nb)
                            nc.vector.tensor_add(eacc, equad, e_sb)
                            equad = eacc
                        if c % 4 == 3:
                            quads.append(equad)
                    rs_defer.append((rs_ps, quads))
                    pending.append((qb, h, qs_sl, pv_ps, rs_ps))
                    if len(pending) > 1:
                        fqb, fh, *rest = pending.pop(0)
                        finalize(fh, *rest)
                        if fh == HG - 1:
                            outproj(fqb)
            for rsd in rs_defer:
                emit_rs(*rsd)
            for fqb, fh, *rest in pending:
                finalize(fh, *rest)
                if fh == HG - 1:
                    outproj(fqb)

    nc.compile()
    _BUILD_CACHE[key] = nc
    return nc


def _host_prep(x, positions, wq, wk, wv, wo):
    """Returns per-core input maps."""
    npdt = _np_mm_dt()

    pos_f = positions.astype(np.float32)
    inv_freq = (
        1.0
        / (ROPE_THETA ** (np.arange(0, HEAD_DIM, 2, dtype=np.float32) / HEAD_DIM))
    ).astype(np.float32)
    ang = pos_f[:, None] * inv_freq[None, :]        # [S, 64]
    csT = np.ascontiguousarray(np.cos(ang).T.astype(np.float32))  # [64, S]
    snT = np.ascontiguousarray(np.sin(ang).T.astype(np.float32))  # [64, S]
    csD = np.concatenate([csT, csT], axis=0)                      # [128, S]
    snS = np.concatenate([-snT, snT], axis=0)                     # [128, S]
    attn_scales = (
        np.log(np.floor((pos_f + 1.0) / FLOOR_SCALE) + 1.0) * ATTN_SCALE + 1.0
    )
    qscale = (attn_scales / np.sqrt(np.float32(HEAD_DIM))).astype(np.float32)
    qsg = np.ones((NRN, SB), np.float32)
    for sb in range(NSB):
        for j in range(HG):
            qsg[4 + sb * HG + j, :] = qscale[sb * SB:(sb + 1) * SB]

    # rotate-half permutation of q/k feature dims (per head), folded into
    # the projection weight columns: permuted feature j<64 <- 2j, j>=64 <- 2(j-64)+1
    perm = np.concatenate([np.arange(0, HEAD_DIM, 2), np.arange(1, HEAD_DIM, 2)])
    wq_p = wq.reshape(D, N_HEADS, HEAD_DIM)[:, :, perm].reshape(D, N_HEADS * HEAD_DIM)
    wk_p = wk[:, perm]

    def tile_x(xT):
        # [D, S] -> [sb, dg, p, c, s]
        return np.ascontiguousarray(
            xT.reshape(8, 4, P, NSB, SB).transpose(3, 0, 2, 1, 4)
        )

    def tile_w(w):
        # [D, m] -> [dg, p, c, m]
        m = w.shape[1]
        return np.ascontiguousarray(
            w.reshape(8, 4, P, m).transpose(0, 2, 1, 3)
        )

    def tile_wo(wg):
        # [256, D] -> [p, hh, cc, q]
        return np.ascontiguousarray(
            wg.reshape(HG, P, NCC, P).transpose(1, 0, 2, 3)
        )

    in_maps = []
    for core in range(8):
        b, g = core // 2, core % 2
        xT = np.ascontiguousarray(x[b].T).astype(npdt, copy=False)
        in_maps.append(
            {
                "xT": tile_x(xT),
                "wq_g": tile_w(
                    wq_p[:, g * HG * HEAD_DIM:(g + 1) * HG * HEAD_DIM].astype(npdt)
                ),
                "wk": tile_w(wk_p.astype(npdt)),
                "wv": tile_w(wv.astype(npdt)),
                "wo_g": tile_wo(
                    wo[g * HG * HEAD_DIM:(g + 1) * HG * HEAD_DIM, :].astype(npdt)
                ),
                "csD": csD,
                "snS": snS,
                "qsg": qsg,
            }
        )
    return in_maps


def kernel(x, positions, wq, wk, wv, wo, _trace=False, _trace_kwargs=None):
    x = np.asarray(x, np.float32)
    positions = np.asarray(positions)
    wq = np.asarray(wq, np.float32)
    wk = np.asarray(wk, np.float32)
    wv = np.asarray(wv, np.float32)
    wo = np.asarray(wo, np.float32)

    nc = build_bass()
    in_maps = _host_prep(x, positions, wq, wk, wv, wo)
    res = bass_utils.run_bass_kernel_spmd(
        nc, in_maps, core_ids=list(range(8)), trace=_trace,
        **(_trace_kwargs or {}),
    )
    kernel.last_results = res

    out = np.empty((B, S, D), np.float32)
    for b in range(B):
        pa = res.results[2 * b]["outT"].astype(np.float32)
        pb = res.results[2 * b + 1]["outT"].astype(np.float32)
        full = (pa + pb).transpose(0, 2, 1, 3).reshape(D, S)
        out[b] = full.T
    return out


# revision 32
# speedup vs baseline: 1.0388x; 1.0388x over previous
"""Trainium2 Bass kernel for nn_Attention_56530359550323.

Full-input contract: kernel(**inputs) takes the unsharded inputs and returns
the full [4, 2048, 4096] float32 output.

Sharding: 8 cores = 4 batches (data-parallel) x 2 head-groups
(tensor-parallel over the 4 query heads; the single kv head is replicated).
Each core computes a partial output-projection [4096, 2048] (transposed);
the host sums the two partials per batch ("all-reduce after wo") and
transposes back.

Device algorithm (everything feature-major / transposed so every matmul has
a 512-wide moving operand, full rate):
  phase 1: qT/kT/vT = W^T @ xT accumulated over 32 d-chunks, with the
           per-d-group weight DMAs interleaved with the x stream so the
           first matmuls start ~5us in.  The rope epilogue for s-block sb
           is emitted after the projection matmuls of sb+1 so its DVE/ACT
           work overlaps PE time: rope is 2 ACT half-swaps + 3
           full-128-partition DVE ops against host-prepared duplicated-cos
           / signed-sin tables.  The 12 per-position sum-of-squares rows
           (4 k + 8 q) are collected into one [12,512] SBUF tile and
           normalized with a single Sqrt -> reciprocal_approx_fast ->
           qscale-mul chain (one pass instead of 12 serial
           single-partition reciprocals).  Q factors are broadcast over
           partitions with K=1 ones matmuls; K factors are NOT applied to
           kT at all -- they are transposed into a [128 kv, 1] per-chunk
           layout and folded into phase 2's Exp as its per-partition
           scale operand (exp(s*f_k) == exp-of-normalized-score).
  phase 2: per (q-block 512, head): scoresT = kT_chunk^T @ qT (chunk=128 kv
           positions), exp on ACT with the per-kv k-norm scale (scores are
           bounded ~6 after qk-norm so no max subtraction is needed),
           causal masking only on the 4 diagonal chunks via constant uint8
           masks, PV accumulation outT += v_chunk^T @ expT, row sums via a
           ones-column matmul, normalization by the broadcast reciprocal
           (approx-fast on SBUF).
  phase 3: partial out-projection outT[cc] = sum_h wo[h,cc]^T @ attnT_h,
           written back as float16 partials (host sums in fp32).
"""

import os
import sys
from contextlib import ExitStack

import numpy as np

if "/opt/trn_rl_repo" not in sys.path:
    sys.path.insert(0, "/opt/trn_rl_repo")

import concourse.bass as bass
import concourse.mybir as mybir
import concourse.tile as tile
from concourse import bacc, bass_utils

# ---- problem constants (hardcoded per contract) ----
B, S, D = 4, 2048, 4096
HEAD_DIM = 128
N_HEADS = 4            # local q heads in the reference module
N_KV = 1
ROPE_THETA = 500000.0
EPS = 1e-6
FLOOR_SCALE = 8192.0
ATTN_SCALE = 0.1

P = 128                # partitions
SB = 512               # s-block (q-block) size
NSB = S // SB          # 4
ND = D // P            # 32 contraction chunks for projections
NKCH = S // P          # 16 kv chunks
NCC = D // P           # 32 output column chunks
HG = 2                 # heads per group (tensor-parallel degree 2)
NRN = 3 * NSB          # rope_norm rows: k rows 0..3, q rows 4..11
NQR = HG * NSB         # 8 q rows

f32 = mybir.dt.float32
f32r = mybir.dt.float32r
f16 = mybir.dt.float16
bf16 = mybir.dt.bfloat16
u8 = mybir.dt.uint8

MM_MODE = os.environ.get("KERNEL_MM_MODE", "f16")

_BUILD_CACHE = {}


def _mm_ap(ap):
    if MM_MODE == "f32r" and ap.dtype == f32:
        return ap.bitcast(f32r)
    return ap


def _dram_mm_dt():
    return {"f32r": f32r, "f16": f16, "bf16": bf16}[MM_MODE]


def _tile_mm_dt():
    return {"f32r": f32r, "f16": f16, "bf16": bf16}[MM_MODE]


def _np_mm_dt():
    if MM_MODE == "f32r":
        return np.float32
    if MM_MODE == "f16":
        return np.float16
    import ml_dtypes

    return ml_dtypes.bfloat16


def build_bass():
    key = MM_MODE
    if key in _BUILD_CACHE:
        return _BUILD_CACHE[key]

    wdt = _dram_mm_dt()      # dram/sbuf dtype for x/w (matmul-only tensors)
    tdt = _tile_mm_dt()      # sbuf dtype for DVE/ACT-produced matmul inputs

    nc = bacc.Bacc("TRN2", target_bir_lowering=False, debug=False)

    # all big tensors arrive pre-tiled host-side so every DMA is a
    # contiguous per-partition read (avoids the 256B-1KB descriptor storm)
    xT_d = nc.dram_tensor("xT", (NSB, 8, P, 4, SB), wdt, kind="ExternalInput").ap()
    wq_d = nc.dram_tensor("wq_g", (8, P, 4, HG * HEAD_DIM), wdt, kind="ExternalInput").ap()
    wk_d = nc.dram_tensor("wk", (8, P, 4, HEAD_DIM), wdt, kind="ExternalInput").ap()
    wv_d = nc.dram_tensor("wv", (8, P, 4, HEAD_DIM), wdt, kind="ExternalInput").ap()
    wo_d = nc.dram_tensor("wo_g", (P, HG, NCC, P), wdt, kind="ExternalInput").ap()
    # duplicated cos [cos; cos] and signed sin [-sin; +sin], feature-major
    csD_d = nc.dram_tensor("csD", (P, S), wdt, kind="ExternalInput").ap()
    snS_d = nc.dram_tensor("snS", (P, S), wdt, kind="ExternalInput").ap()
    # per-norm-row scale grid: k rows (0..3) = 1, q rows (4..11) = qscale
    qsg_d = nc.dram_tensor("qsg", (NRN, SB), f32, kind="ExternalInput").ap()
    out_d = nc.dram_tensor("outT", (NCC, NSB, P, SB), f16, kind="ExternalOutput").ap()

    # masks for the 4 diagonal chunks of a 512-q block: 1 => future (kill)
    masks_np = np.zeros((P, 4, SB), np.uint8)
    for c in range(4):
        kp = c * P + np.arange(P)[:, None]
        qf = np.arange(SB)[None, :]
        masks_np[:, c, :] = (kp > qf).astype(np.uint8)
    masks_d = nc.inline_tensor(masks_np, name="cmasks")
    ident_d = nc.inline_tensor(np.eye(P, dtype=_np_mm_dt()), name="ident")
    ident4_d = nc.inline_tensor(np.eye(4, dtype=_np_mm_dt()), name="ident4")
    # esel[:, r, j] = (j == r): ones-column matmul that drops the sum-of-
    # squares of row r into partition r of a shared [12,512] PSUM tile
    esel_np = np.broadcast_to(
        np.eye(NRN, dtype=_np_mm_dt()), (P, NRN, NRN)
    ).copy()
    esel_d = nc.inline_tensor(esel_np, name="esel")
    # qsel[p, r, :] = (p == 4 + r): broadcasts row 4+r of the norm-factor
    # tile to all 128 partitions in a single K=12 matmul
    qsel_np = np.zeros((NRN, NQR, P), _np_mm_dt())
    for r in range(NQR):
        qsel_np[4 + r, r, :] = 1
    qsel_d = nc.inline_tensor(qsel_np, name="qsel")

    Exp = mybir.ActivationFunctionType.Exp
    Sqrt = mybir.ActivationFunctionType.Sqrt
    Square = mybir.ActivationFunctionType.Square

    with tile.TileContext(nc) as tc, ExitStack() as top:
        cpool = top.enter_context(tc.tile_pool(name="consts", bufs=1))
        qkpool = top.enter_context(tc.tile_pool(name="qkv", bufs=1))
        atpool = top.enter_context(tc.tile_pool(name="attn", bufs=1))

        # const tiles (DMA posts deferred into the phase-1 loop so the
        # critical first weight/x loads hit the queues first)
        masks_t = cpool.tile([P, 4, SB], u8)
        ident_t = cpool.tile([P, P], tdt)
        ident4_t = cpool.tile([4, 4], tdt)
        esel_t = cpool.tile([P, NRN, NRN], tdt)
        qsel_t = cpool.tile([NRN, NQR, P], tdt)
        onesrow_t = cpool.tile([1, P], tdt)
        nc.vector.memset(onesrow_t, 1.0)
        onescol_t = cpool.tile([P, 1], tdt)
        nc.vector.memset(onescol_t, 1.0)
        zero_t = cpool.tile([P, SB], tdt)
        nc.vector.memset(zero_t, 0.0)
        eps12_t = cpool.tile([NRN, 1], f32)
        nc.vector.memset(eps12_t, float(EPS))

        # cross-phase SBUF handoff tiles
        qT_t = qkpool.tile([P, HG, S], tdt)       # normed+roped+scaled qT
        kT_t = qkpool.tile([P, S], tdt)           # roped (UN-normed) kT
        fkT_t = qkpool.tile([P, NSB, 4], f32)     # k norm factor, kv-partition-major
        vnat_t = qkpool.tile([P, NKCH, P], tdt)   # v in natural [s, hd] tiles
        attnT_t = atpool.tile([P, HG, S], tdt)

        # ---------------- phase 1: projections ----------------
        with ExitStack() as ph1:
            rcpool = ph1.enter_context(tc.tile_pool(name="ropec", bufs=1))
            wpool = ph1.enter_context(tc.tile_pool(name="projw", bufs=1))
            xpool = ph1.enter_context(tc.tile_pool(name="xstream", bufs=4))
            epool = ph1.enter_context(tc.tile_pool(name="ep1", bufs=2))
            tpool = ph1.enter_context(tc.tile_pool(name="ropetmp", bufs=2))
            p1ps = ph1.enter_context(tc.tile_pool(name="p1ps", bufs=1, space="PSUM"))
            p1ps2 = ph1.enter_context(tc.tile_pool(name="p1ps2", bufs=1, space="PSUM"))

            csD_t = rcpool.tile([P, S], tdt)
            snS_t = rcpool.tile([P, S], tdt)
            qsg_t = rcpool.tile([NRN, SB], f32)
            ropes = rcpool.tile([P, NQR, SB], tdt)  # roped (un-normed) q rows
            # all 12 sum-of-squares rows accumulate into one PSUM tile
            ss_all_ps = p1ps2.tile([NRN, SB], f32, tag="ss")
            ss_count = [0]

            wq_t = wpool.tile([P, ND, HG * HEAD_DIM], wdt)
            wk_t = wpool.tile([P, ND, HEAD_DIM], wdt)
            wv_t = wpool.tile([P, ND, HEAD_DIM], wdt)

            def epilogue_a(sb, copies, kc, v_sb):
                """rope + square + sum-of-squares for s-block sb; v transpose."""
                ss = slice(sb * SB, (sb + 1) * SB)
                for j, srcc in enumerate((copies[0], copies[1], kc)):
                    is_k = j == 2
                    r = sb if is_k else 4 + sb * HG + j
                    rope_dst = kT_t[:, ss] if is_k else ropes[:, r - 4, :]
                    swp = tpool.tile([P, SB], tdt, tag="swp")
                    nc.scalar.copy(swp[0:64, :], srcc[64:128, :])
                    nc.scalar.copy(swp[64:128, :], srcc[0:64, :])
                    t1 = tpool.tile([P, SB], tdt, tag="t1")
                    nc.vector.tensor_mul(t1, srcc, csD_t[:, ss])
                    t2 = tpool.tile([P, SB], tdt, tag="t2")
                    nc.vector.tensor_mul(t2, swp, snS_t[:, ss])
                    nc.vector.tensor_add(rope_dst, t1, t2)
                    sq = tpool.tile([P, SB], tdt, tag="sq")
                    nc.scalar.activation(sq, rope_dst, Square)
                    nc.tensor.matmul(
                        ss_all_ps, _mm_ap(esel_t[:, r, :]), _mm_ap(sq[:]),
                        start=(ss_count[0] == 0),
                        stop=(ss_count[0] == NRN - 1),
                    )
                    ss_count[0] += 1
                for t in range(4):
                    tp_ps = p1ps2.tile([P, P], tdt, tag="tp")
                    nc.tensor.transpose(tp_ps, v_sb[:, t * P:(t + 1) * P], _mm_ap(ident_t[:]))
                    nc.vector.tensor_copy(vnat_t[:, sb * 4 + t, :], tp_ps)

            deferred = []
            for sb in range(NSB):
                q_ps = [
                    p1ps.tile([P, SB], f32, tag=f"q{h}", name=f"qps{h}")
                    for h in range(HG)
                ]
                k_ps = p1ps.tile([P, SB], f32, tag="k")
                v_ps = p1ps.tile([P, SB], f32, tag="v")
                for dg in range(8):
                    dsl = slice(dg * 4, (dg + 1) * 4)
                    if sb == 0:
                        # interleave weight loads with the x stream so the
                        # first d-group's matmuls start ~5us in
                        nc.sync.dma_start(wq_t[:, dsl, :], wq_d[dg])
                        nc.scalar.dma_start(wk_t[:, dsl, :], wk_d[dg])
                        nc.gpsimd.dma_start(wv_t[:, dsl, :], wv_d[dg])
                        if dg == 1:
                            nc.gpsimd.dma_start(csD_t, csD_d)
                            nc.gpsimd.dma_start(snS_t, snS_d)
                        elif dg == 2:
                            nc.scalar.dma_start(esel_t, esel_d.ap())
                            nc.scalar.dma_start(ident_t, ident_d.ap())
                            nc.scalar.dma_start(ident4_t, ident4_d.ap())
                            nc.gpsimd.dma_start(masks_t, masks_d.ap())
                        elif dg == 3:
                            nc.gpsimd.dma_start(qsg_t, qsg_d)
                            nc.scalar.dma_start(qsel_t, qsel_d.ap())
                    xt = xpool.tile([P, 4, SB], wdt)
                    if sb == 0 and dg == 0:
                        # split the first x tile so the first matmul's
                        # operand lands as early as possible
                        for c4 in range(4):
                            nc.sync.dma_start(
                                xt[:, c4, :], xT_d[sb, dg, :, c4, :]
                            )
                    else:
                        nc.sync.dma_start(xt, xT_d[sb, dg])
                    for c in range(4):
                        d = dg * 4 + c
                        st, sp = (d == 0), (d == ND - 1)
                        rhs = _mm_ap(xt[:, c, :])
                        for h in range(HG):
                            nc.tensor.matmul(
                                q_ps[h],
                                _mm_ap(wq_t[:, d, h * P:(h + 1) * P]),
                                rhs,
                                start=st,
                                stop=sp,
                            )
                        nc.tensor.matmul(
                            k_ps, _mm_ap(wk_t[:, d, :]), rhs, start=st, stop=sp
                        )
                        nc.tensor.matmul(
                            v_ps, _mm_ap(wv_t[:, d, :]), rhs, start=st, stop=sp
                        )

                # free the accumulator banks fast (ACT reads PSUM at line rate)
                copies = []
                for h in range(HG):
                    qc = epool.tile([P, SB], tdt, tag=f"qc{h}", bufs=2,
                                    name=f"qcopy{h}")
                    nc.scalar.copy(qc, q_ps[h])
                    copies.append(qc)
                kc = epool.tile([P, SB], tdt, tag="kc", bufs=2)
                nc.scalar.copy(kc, k_ps)
                v_sb = epool.tile([P, SB], tdt, tag="vc", bufs=2)
                nc.vector.tensor_copy(v_sb, v_ps)
                deferred.append((sb, copies, kc, v_sb))
                # emit the previous block's epilogue after this block's
                # matmuls: its DVE/ACT work overlapped this block's PE time
                if len(deferred) > 1:
                    epilogue_a(*deferred.pop(0))
            epilogue_a(*deferred.pop(0))

            # batched normalization chain: one pass over all 12 rows
            sqr_all = epool.tile([NRN, SB], f32, tag="sqr", bufs=1)
            nc.scalar.activation(
                sqr_all, ss_all_ps, Sqrt, bias=eps12_t[:], scale=1.0 / HEAD_DIM
            )
            rec_all = epool.tile([NRN, SB], f32, tag="reca", bufs=1)
            nc.vector.reciprocal_approx_fast(rec_all, sqr_all)
            recq_all = epool.tile([NRN, SB], f32, tag="recq", bufs=1)
            nc.vector.tensor_mul(recq_all, rec_all, qsg_t)
            recr_all = epool.tile([NRN, SB], tdt, tag="recr", bufs=1)
            nc.vector.tensor_copy(recr_all, recq_all)

            # k factors -> [128 kv, 1] per-chunk layout via 4 PE transposes
            for m in range(4):
                tp_ps = p1ps2.tile([P, P], tdt, tag="tp")
                nc.tensor.transpose(
                    tp_ps[:, 0:4], recr_all[0:4, m * P:(m + 1) * P], ident4_t[:]
                )
                nc.vector.tensor_copy(fkT_t[:, :, m], tp_ps[:, 0:4])

            # q factors: selector matmul broadcasts row 4+r to all
            # partitions in one shot (reads the full base-0 tile)
            for r in range(NQR):
                sb, j = divmod(r, HG)
                ss = slice(sb * SB, (sb + 1) * SB)
                bc_ps = p1ps2.tile([P, SB], f32, tag="bc", bufs=2)
                nc.tensor.matmul(
                    bc_ps, _mm_ap(qsel_t[:, r, :]), _mm_ap(recr_all[:]),
                    start=True, stop=True,
                )
                bcs = epool.tile([P, SB], f32, tag="bcs", bufs=2)
                nc.scalar.copy(bcs, bc_ps)
                nc.vector.tensor_mul(qT_t[:, j, ss], ropes[:, r, :], bcs)

        # ---------------- phase 2: attention ----------------
        with ExitStack() as ph2:
            e2pool = ph2.enter_context(tc.tile_pool(name="ep2", bufs=2))
            wopool = ph2.enter_context(tc.tile_pool(name="wo", bufs=1))
            o3pool = ph2.enter_context(tc.tile_pool(name="oc", bufs=8))
            expool = ph2.enter_context(tc.tile_pool(name="exps", bufs=12))
            p2ps = ph2.enter_context(tc.tile_pool(name="p2ps", bufs=2, space="PSUM"))
            p2sc = ph2.enter_context(tc.tile_pool(name="p2sc", bufs=1, space="PSUM"))

            def finalize(h, qs_sl, pv_ps, rs_ps):
                # normalization chain, emitted one group late so its PE
                # broadcast matmul never stalls the in-order PE stream.
                pvs = e2pool.tile([P, SB], f32, tag="pvs")
                nc.scalar.copy(pvs, pv_ps)
                rss = e2pool.tile([1, SB], f32, tag="rss")
                nc.scalar.copy(rss, rs_ps)
                rec = e2pool.tile([1, SB], f32, tag="rec")
                nc.vector.reciprocal_approx_fast(rec, rss)
                recr = e2pool.tile([1, SB], tdt, tag="recr")
                nc.vector.tensor_copy(recr, rec)
                bc_ps = p2sc.tile([P, SB], f32, tag="obc", bufs=3)
                nc.tensor.matmul(
                    bc_ps, _mm_ap(onesrow_t[:]), _mm_ap(recr[:]),
                    start=True, stop=True,
                )
                bc_sb = e2pool.tile([P, SB], f32, tag="bcc")
                nc.vector.tensor_copy(bc_sb, bc_ps)
                nc.vector.tensor_mul(attnT_t[:, h, qs_sl], pvs, bc_sb)

            wo_t = wopool.tile([P, HG, NCC, P], wdt)
            nc.sync.dma_start(wo_t, wo_d)

            def outproj(qb):
                qsl = slice(qb * SB, (qb + 1) * SB)
                for cc in range(NCC):
                    o_ps = p2sc.tile([P, SB], f32, tag="obc", bufs=3, name="ops")
                    for h in range(HG):
                        nc.tensor.matmul(
                            o_ps,
                            _mm_ap(wo_t[:, h, cc, :]),
                            _mm_ap(attnT_t[:, h, qsl]),
                            start=(h == 0),
                            stop=(h == HG - 1),
                        )
                    o_sb = o3pool.tile([P, SB], f16, tag="oc")
                    if cc % 2 == 0:
                        nc.vector.tensor_copy(o_sb, o_ps)
                        nc.gpsimd.dma_start(out_d[cc, qb], o_sb)
                    else:
                        nc.scalar.copy(o_sb, o_ps)
                        nc.sync.dma_start(out_d[cc, qb], o_sb)

            def emit_rs(rs_ps, quads):
                # row-sum matmuls for a finished group: its quad sums are
                # long done, so these never stall the PE stream
                for i, eq in enumerate(quads):
                    nc.tensor.matmul(
                        rs_ps, _mm_ap(onescol_t[:]), _mm_ap(eq[:]),
                        start=(i == 0), stop=(i == len(quads) - 1),
                    )

            pending = []
            rs_defer = []
            for qb in range(NSB):
                qs_sl = slice(qb * SB, (qb + 1) * SB)
                nch = 4 * qb + 4
                for h in range(HG):
                    pv_ps = p2ps.tile([P, SB], f32, tag="pv")
                    rs_ps = p2ps.tile([1, SB], f32, tag="rs", bufs=1)
                    qt = qT_t[:, h, qs_sl]
                    if rs_defer:
                        emit_rs(*rs_defer.pop(0))
                    equad = None
                    quads = []
                    exps = []

                    def emit_sc(c):
                        # scores + exp for chunk c, staggered one chunk ahead
                        # of its PV matmul so the PE never waits on the ACT
                        sc_ps = p2sc.tile([P, SB], f32, tag="sc", bufs=2)
                        nc.tensor.matmul(
                            sc_ps,
                            _mm_ap(kT_t[:, c * P:(c + 1) * P]),
                            _mm_ap(qt),
                            start=True,
                            stop=True,
                        )
                        e_sb = expool.tile([P, SB], tdt, tag="exp")
                        nc.scalar.activation(
                            e_sb, sc_ps, Exp, scale=fkT_t[:, c // 4, c % 4:c % 4 + 1]
                        )
                        if c >= 4 * qb:
                            nc.vector.copy_predicated(
                                e_sb, masks_t[:, c - 4 * qb, :], zero_t
                            )
                        exps.append(e_sb)

                    emit_sc(0)
                    for c in range(nch):
                        if c + 1 < nch:
                            emit_sc(c + 1)
                        e_sb = exps[c]
                        st, sp = (c == 0), (c == nch - 1)
                        nc.tensor.matmul(
                            pv_ps, _mm_ap(vnat_t[:, c, :]), _mm_ap(e_sb[:]),
                            start=st, stop=sp,
                        )
                        # accumulate quads of exp tiles on DVE so the row-sum
                        # matmul runs once per 4 chunks instead of per chunk
                        if c % 4 == 0:
                            equad = e_sb
                        else:
                            nb = 8 if c % 4 == 3 else 2
                            eacc = expool.tile([P, SB], tdt, tag=f"ea{c % 4}",
                                               bufs=nb)
                            nc.vector.tensor_add(eacc, equad, e_sb)
                            equad = eacc
                        if c % 4 == 3:
                            quads.append(equad)
                    rs_defer.append((rs_ps, quads))
                    pending.append((qb, h, qs_sl, pv_ps, rs_ps))
                    if len(pending) > 1:
                        fqb, fh, *rest = pending.pop(0)
                        finalize(fh, *rest)
                        if fh == HG - 1:
                            outproj(fqb)
            for rsd in rs_defer:
                emit_rs(*rsd)
            for fqb, fh, *rest in pending:
                finalize(fh, *rest)
                if fh == HG - 1:
                    outproj(fqb)

    nc.compile()
    _BUILD_CACHE[key] = nc
    return nc


def _host_prep(x, positions, wq, wk, wv, wo):
    """Returns per-core input maps."""
    npdt = _np_mm_dt()

    pos_f = positions.astype(np.float32)
    inv_freq = (
        1.0
        / (ROPE_THETA ** (np.arange(0, HEAD_DIM, 2, dtype=np.float32) / HEAD_DIM))
    ).astype(np.float32)
    ang = pos_f[:, None] * inv_freq[None, :]        # [S, 64]
    csT = np.ascontiguousarray(np.cos(ang).T.astype(np.float32))  # [64, S]
    snT = np.ascontiguousarray(np.sin(ang).T.astype(np.float32))  # [64, S]
    csD = np.concatenate([csT, csT], axis=0).astype(npdt)         # [128, S]
    snS = np.concatenate([-snT, snT], axis=0).astype(npdt)        # [128, S]
    attn_scales = (
        np.log(np.floor((pos_f + 1.0) / FLOOR_SCALE) + 1.0) * ATTN_SCALE + 1.0
    )
    qscale = (attn_scales / np.sqrt(np.float32(HEAD_DIM))).astype(np.float32)
    qsg = np.ones((NRN, SB), np.float32)
    for sb in range(NSB):
        for j in range(HG):
            qsg[4 + sb * HG + j, :] = qscale[sb * SB:(sb + 1) * SB]

    # rotate-half permutation of q/k feature dims (per head), folded into
    # the projection weight columns: permuted feature j<64 <- 2j, j>=64 <- 2(j-64)+1
    perm = np.concatenate([np.arange(0, HEAD_DIM, 2), np.arange(1, HEAD_DIM, 2)])
    wq_p = wq.reshape(D, N_HEADS, HEAD_DIM)[:, :, perm].reshape(D, N_HEADS * HEAD_DIM)
    wk_p = wk[:, perm]

    def tile_x(xT):
        # [D, S] -> [sb, dg, p, c, s]
        return np.ascontiguousarray(
            xT.reshape(8, 4, P, NSB, SB).transpose(3, 0, 2, 1, 4)
        )

    def tile_w(w):
        # [D, m] -> [dg, p, c, m]
        m = w.shape[1]
        return np.ascontiguousarray(
            w.reshape(8, 4, P, m).transpose(0, 2, 1, 3)
        )

    def tile_wo(wg):
        # [256, D] -> [p, hh, cc, q]
        return np.ascontiguousarray(
            wg.reshape(HG, P, NCC, P).transpose(1, 0, 2, 3)
        )

    in_maps = []
    for core in range(8):
        b, g = core // 2, core % 2
        xT = np.ascontiguousarray(x[b].T).astype(npdt, copy=False)
        in_maps.append(
            {
                "xT": tile_x(xT),
                "wq_g": tile_w(
                    wq_p[:, g * HG * HEAD_DIM:(g + 1) * HG * HEAD_DIM].astype(npdt)
                ),
                "wk": tile_w(wk_p.astype(npdt)),
                "wv": tile_w(wv.astype(npdt)),
                "wo_g": tile_wo(
                    wo[g * HG * HEAD_DIM:(g + 1) * HG * HEAD_DIM, :].astype(npdt)
                ),
                "csD": csD,
                "snS": snS,
                "qsg": qsg,
            }
        )
    return in_maps


def kernel(x, positions, wq, wk, wv, wo, _trace=False, _trace_kwargs=None):
    x = np.asarray(x, np.float32)
    positions = np.asarray(positions)
    wq = np.asarray(wq, np.float32)
    wk = np.asarray(wk, np.float32)
    wv = np.asarray(wv, np.float32)
    wo = np.asarray(wo, np.float32)

    nc = build_bass()
    in_maps = _host_prep(x, positions, wq, wk, wv, wo)
    res = bass_utils.run_bass_kernel_spmd(
        nc, in_maps, core_ids=list(range(8)), trace=_trace,
        **(_trace_kwargs or {}),
    )
    kernel.last_results = res

    out = np.empty((B, S, D), np.float32)
    for b in range(B):
        pa = res.results[2 * b]["outT"].astype(np.float32)
        pb = res.results[2 * b + 1]["outT"].astype(np.float32)
        full = (pa + pb).transpose(0, 2, 1, 3).reshape(D, S)
        out[b] = full.T
    return out
